# revision 54
# baseline (speedup 1.0000x reference)
"""Local (bucketed) attention Bass kernel for Trainium2, 8 NeuronCores SPMD.

Problem (hardcoded): B=8, H=8, T=8192, E=64, BUCKETS=128, bucket=64,
look_backward=1, look_forward=0, causal, no 1/sqrt(E) scaling.

Sharding: batch*heads (64) split across 8 cores -> 8 bh per core,
processed as 4 "pairs"; within a pair, bh 2p is "stream A" (SBUF
partitions 0..63 of q/k) and bh 2p+1 is "stream B" (64..127).

Design (2.3x the session-start baseline; sim/HW-verified):
  - q/k shipped fp16 (single-pass mm1, ~8x the mantissa of bf16 at the
    same byte cost); v bf16; out fp16 + f32 host fixup.
  - mm1 per (stream, window w): stationary = kt 2-bucket slab in
    "parity order" (even-index bucket of the window in PE rows 0..63,
    odd in 64..127), moving = qt bucket w -> dotsT [128 j, 64 q].
    Odd w: one matmul (ascending slab is contiguous).  Even w: two
    standalone matmuls (HW requires stationary APs with ONE free dim,
    so the descending slab is not expressible).
  - exp on ACT into bf16; causal+window mask applied multiplicatively
    on DVE (2/5) / GPSIMD (3/5) with parity masks [tri;ones]/[ones;tri].
  - v parity layout va[128 j, s, m, e]: partition j<64 = bucket 2m,
    j>=64 = bucket 2m+1.  mm2 odd w: one K=128 matmul into the main
    PSUM slot.  Even w: matmuls accumulating into one PSUM region MUST
    share the lhsT partition range on HW (row-switch accumulation is an
    NRT_EXEC_UNIT_UNRECOVERABLE), so the cur half goes to the main slot
    and the prev half to an aux slot; the host adds aux terms during
    unpack.  Row sums via 1-column ones-matmuls; out tile layout per
    group = 8 main + 4 aux + 12 r columns in one 2-bank PSUM tile.
  - reciprocal + broadcast-normalize on DVE; input DMAs quarter-chunked
    and dripped one-per-unit across the SP and GPSIMD queues (the cost
    model charges DMA transfer time to the issuing engine, and a big
    DMA at a queue head blocks latency-critical work behind it);
    outputs on SP.  Software pipeline: mm2 trails mm1 by 3 units
    (PE FIFO head-of-line cover for the exp->mask round trip), units
    alternate between the two resident pairs, all 4 pairs' tiles
    resident (bufs=4) so block transitions don't stall.

HW constraints discovered (CoreSim does not model them):
  - matmul stationary AP must collapse to a single free dimension
    (no negative-stride or broadcast 2-bucket slabs).
  - all matmuls of one PSUM accumulation group (including
    start=False standalone accumulates) must use the same lhsT/rhs
    partition range; different tile_position rows crash the PE.
  - GPSIMD (Pool) tensor ops cannot access PSUM.
"""

import numpy as np
import ml_dtypes

BH_PER_CORE = 8
N_CORES = 8
T = 8192
E = 64
BS = 64  # bucket size
NBUCK = T // BS  # 128
NPAIR = BH_PER_CORE // 2  # 4
SGRP = 8   # windows per compute group
OGRP = 1   # compute groups per output DMA

MM1_MODE = "fp16-k128"  # informational only

_PROGRAM_CACHE = {}


def _build_program(n_pairs=NPAIR, nbuck=NBUCK):
    import concourse.bass as bass
    import concourse.tile as tile
    from concourse import bacc, mybir

    F32 = mybir.dt.float32
    F16 = mybir.dt.float16
    BF16 = mybir.dt.bfloat16
    Exp = mybir.ActivationFunctionType.Exp
    mult = mybir.AluOpType.mult

    nc = bacc.Bacc("TRN2", target_bir_lowering=False, debug=False,
                   num_devices=N_CORES)

    seqlen = nbuck * BS  # 8192
    # merged input: per partition fp16[0:8192]=qT, fp16[8192:16384]=kT,
    # bf16-bits[16384:24576]=va (s-major: s, m, e)
    qkv_d = nc.dram_tensor("qkv", [n_pairs, 128, 3 * seqlen], F16,
                           kind="ExternalInput").ap()
    # 12 slots per group: 8 normalized window outputs + 4 aux terms
    # (even windows' prev-half contributions; host adds them in)
    out_d = nc.dram_tensor("out", [n_pairs, 128, nbuck // SGRP, 12, BS], F16,
                           kind="ExternalOutput").ap()

    # Masks, [128 j, parity, 64 i]:
    #   parity 0 (even w): cur bucket on rows 0..63 -> [tri; ones]
    #   parity 1 (odd  w): cur bucket on rows 64..127 -> [ones; tri]
    tri = (np.arange(BS)[:, None] <= np.arange(BS)[None, :])  # keep j<=i
    mask_np = np.empty((128, 2, BS), dtype=ml_dtypes.bfloat16)
    mask_np[0:64, 0] = tri.astype(ml_dtypes.bfloat16)
    mask_np[64:128, 0] = 1.0
    mask_np[0:64, 1] = 1.0
    mask_np[64:128, 1] = tri.astype(ml_dtypes.bfloat16)
    mask_dram = nc.inline_tensor(np.ascontiguousarray(mask_np), name="winmask")
    ones_dram = nc.inline_tensor(
        np.ones((128, 1), dtype=ml_dtypes.bfloat16), name="onescol")

    ngrp = nbuck // SGRP  # 16

    with tile.TileContext(nc) as tc:
        with (
            tc.tile_pool(name="consts", bufs=1) as consts,
            tc.tile_pool(name="qkv", bufs=4) as qkvp,
            tc.tile_pool(name="expp", bufs=5) as expp,
            tc.tile_pool(name="outsb", bufs=3) as outsbp,
            tc.tile_pool(name="rp", bufs=3) as rp,
            tc.tile_pool(name="fat", bufs=2, space="PSUM") as fatp,
            tc.tile_pool(name="outr", bufs=2, space="PSUM") as outrp,
        ):
            mask_sb = consts.tile([128, 2, BS], BF16)
            nc.sync.dma_start(mask_sb[:], mask_dram.ap())
            ones_sb = consts.tile([128, 1], BF16)
            nc.sync.dma_start(ones_sb[:], ones_dram.ap())

            # per-pair views, filled lazily when the pair's DMA is issued
            views = {}

            def _mk_tiles(p):
                kt_sb = qkvp.tile([128, seqlen], F16, tag="kt", name="kt_sb")
                qt_sb = qkvp.tile([128, seqlen], F16, tag="qt", name="qt_sb")
                va_sb = qkvp.tile([128, seqlen], F16, tag="va", name="va_sb")
                qt_r = qt_sb[:].rearrange("p (b x) -> p b x", x=BS)
                kt_r = kt_sb[:].rearrange("p (b x) -> p b x", x=BS)
                # va bits are m-major: [m, s, e] per partition
                va_r = va_sb[:].bitcast(BF16).rearrange(
                    "p (m s e) -> p s m e", s=2, e=BS)
                views[p] = (qt_r, kt_r, va_r)
                return qt_sb, kt_sb, va_sb

            _drip = {0: [], 1: []}  # queue -> list of (tile_slice, dram_slice)

            def queue_pair_chunks(p, nchunk=4):
                # quarter-chunks of kt, qt, va appended to the two drip
                # queues (kt/qt lead va by construction order)
                qt_sb, kt_sb, va_sb = _mk_tiles(p)
                plan = [(kt_sb, seqlen), (qt_sb, 0), (va_sb, 2 * seqlen)]
                qc = seqlen // nchunk
                qi = p % 2
                for ci in range(nchunk):
                    lo = ci * qc
                    for tl, base in plan:
                        _drip[qi].append(
                            (tl[:, lo:lo + qc],
                             qkv_d[p, :, base + lo:base + lo + qc]))
                        qi ^= 1

            def drip(n=1):
                qs = (nc.sync, nc.gpsimd)
                for qi in (0, 1):
                    for _ in range(n):
                        if _drip[qi]:
                            tl, dr = _drip[qi].pop(0)
                            qs[qi].dma_start(tl, dr)

            def load_block0_start():
                t0 = _mk_tiles(0)
                t1 = _mk_tiles(1)
                tensors = [
                    (0, 0, t0[1], seqlen),       # SP:   kt0
                    (1, 0, t0[0], 0),            # Pool: qt0
                    (1, 1, t1[1], seqlen),       # Pool: kt1
                    (0, 1, t1[0], 0),            # SP:   qt1
                    (1, 0, t0[2], 2 * seqlen),   # Pool: va0
                    (0, 1, t1[2], 2 * seqlen),   # SP:   va1
                ]
                # first quarter of everything lands up front (two eighth
                # rounds for kt/qt, one quarter for va)
                qs = (nc.sync, nc.gpsimd)
                e8 = seqlen // 8
                for lo, hi in ((0, e8), (e8, 2 * e8)):
                    for qi, pr, tl, base in tensors[:4]:
                        qs[qi].dma_start(tl[:, lo:hi],
                                         qkv_d[pr, :, base + lo:base + hi])
                for qi, pr, tl, base in tensors[4:]:
                    qs[qi].dma_start(tl[:, 0:2 * e8],
                                     qkv_d[pr, :, base:base + 2 * e8])
                # remaining 3/4, quarter-chunks into the drip queues
                qc = seqlen // 4
                for ci in range(1, 4):
                    lo = ci * qc
                    for qi, pr, tl, base in tensors:
                        _drip[qi].append(
                            (tl[:, lo:lo + qc],
                             qkv_d[pr, :, base + lo:base + lo + qc]))

            def emit_m1(p, g):
                """mm1 group -> fat psum tile; returns (fat, exp_sb)."""
                qt_r, kt_r, va_r = views[p]
                w0 = g * SGRP
                fat = fatp.tile([128, 2, SGRP, BS], F32, tag="fat")
                for s in range(2):
                    sp = s * 64
                    for widx in range(SGRP):
                        w = w0 + widx
                        rhs = qt_r[sp:sp + 64, w, :]
                        if w == 0 or w % 2 == 1:
                            # ascending 2-bucket slab (contiguous, single
                            # free dim).  For w=0 rows 64..127 hold
                            # never-read bucket-1 scores (finite filler).
                            wl = max(w - 1, 0)
                            nc.tensor.matmul(
                                fat[:, s, widx, :],
                                lhsT=kt_r[sp:sp + 64, wl:wl + 2, :],
                                rhs=rhs, start=True, stop=True)
                        else:
                            # even w: cur bucket -> rows 0..63, prev ->
                            # rows 64..127 (HW stationary APs must be a
                            # single free dim, so two matmuls)
                            nc.tensor.matmul(
                                fat[0:64, s, widx, :],
                                lhsT=kt_r[sp:sp + 64, w, :],
                                rhs=rhs, start=True, stop=True)
                            nc.tensor.matmul(
                                fat[64:128, s, widx, :],
                                lhsT=kt_r[sp:sp + 64, w - 1, :],
                                rhs=rhs, start=True, stop=True)
                return fat

            def emit_expmask(p, g, fat, t):
                exp_sb = expp.tile([128, 2, SGRP, BS], BF16, tag="exp")
                nc.scalar.activation(exp_sb[:], fat[:], Exp)
                ev = exp_sb[:].rearrange("p s (a q) i -> p s a q i", q=2)
                eng = nc.vector if t % 5 < 2 else nc.gpsimd
                eng.tensor_tensor(
                    ev, ev,
                    mask_sb[:, None, None, :, :].to_broadcast(
                        (128, 2, SGRP // 2, 2, BS)),
                    mult,
                )
                return exp_sb

            def emit_m2(p, g, exp_sb):
                # outr layout (f32 cols): [0:512) = 8 main slots,
                # [512:768) = 4 aux slots (even windows' prev halves),
                # [768:780) = 12 row-sum columns.  All matmuls standalone
                # or same-partition-range groups (HW requirement).
                qt_r, kt_r, va_r = views[p]
                w0 = g * SGRP
                outr = outrp.tile([128, 784], F32, tag="outr")
                main = outr[:, 0:512].rearrange("p (w e) -> p w e", e=BS)
                aux = outr[:, 512:768].rearrange("p (w e) -> p w e", e=BS)
                rcol = outr[:, 768:780]
                for s in range(2):
                    ob0 = s * 64
                    for widx in range(SGRP):
                        w = w0 + widx
                        ex = exp_sb[:, s, widx, :]
                        o = main[ob0:ob0 + 64, widx, :]
                        ro = rcol[ob0:ob0 + 64, widx:widx + 1]
                        if w == 0:
                            nc.tensor.matmul(
                                o, lhsT=ex[0:64, :],
                                rhs=va_r[0:64, s, 0, :],
                                start=True, stop=True)
                            nc.tensor.matmul(
                                ro, lhsT=ex[0:64, :],
                                rhs=ones_sb[0:64, :],
                                start=True, stop=True)
                            # fill aux slot 0 / r col 8 so the batched
                            # normalize reads initialized psum (host
                            # ignores aux for w=0)
                            nc.tensor.matmul(
                                aux[ob0:ob0 + 64, 0, :], lhsT=ex[0:64, :],
                                rhs=va_r[0:64, s, 0, :],
                                start=True, stop=True)
                            nc.tensor.matmul(
                                rcol[ob0:ob0 + 64, 8:9], lhsT=ex[0:64, :],
                                rhs=ones_sb[0:64, :],
                                start=True, stop=True)
                        elif w % 2 == 1:
                            m = (w - 1) // 2
                            nc.tensor.matmul(
                                o, lhsT=ex, rhs=va_r[:, s, m, :],
                                start=True, stop=True)
                            nc.tensor.matmul(
                                ro, lhsT=ex, rhs=ones_sb[:],
                                start=True, stop=True)
                        else:
                            m = w // 2
                            a = widx // 2
                            nc.tensor.matmul(
                                o, lhsT=ex[0:64, :],
                                rhs=va_r[0:64, s, m, :],
                                start=True, stop=True)
                            nc.tensor.matmul(
                                aux[ob0:ob0 + 64, a, :],
                                lhsT=ex[64:128, :],
                                rhs=va_r[64:128, s, m - 1, :],
                                start=True, stop=True)
                            nc.tensor.matmul(
                                ro, lhsT=ex, rhs=ones_sb[:],
                                start=True, stop=True)
                            nc.tensor.matmul(
                                rcol[ob0:ob0 + 64, 8 + a:9 + a],
                                lhsT=ex, rhs=ones_sb[:],
                                start=True, stop=True)
                return outr

            obs = {}
            _oblk = [0]

            def emit_norm(p, g, outr, scale_idx):
                r_sb = rp.tile([128, 12], F32, tag="r")
                nc.vector.reciprocal(r_sb[:], outr[:, 768:780])
                key = p % 2
                if obs.get(key) is None:
                    ob = outsbp.tile([128, OGRP, 12, BS], F16, tag="ob",
                                     name="ob")
                    obs[key] = ob
                ob = obs[key]
                gslot = g % OGRP
                ov = outr[:, 0:768].rearrange("p (w e) -> p w e", e=BS)
                nc.vector.tensor_tensor(
                    ob[:, gslot, :, :],
                    ov,
                    r_sb[:, :, None].to_broadcast((128, 12, BS)),
                    mult,
                )
                if gslot == OGRP - 1:
                    glo = g - OGRP + 1
                    eng = nc.sync
                    _oblk[0] += 1
                    eng.dma_start(out_d[p, :, glo:glo + OGRP, :, :], ob[:])
                    obs[key] = None

            # flat unit list: two pair-blocks, pairs interleaved inside
            units = []
            for blk in range(n_pairs // 2):
                for g in range(ngrp):
                    for pp in range(2):
                        units.append((2 * blk + pp, g))

            load_block0_start()
            DSKEW = 3
            pending = []  # [(p, g, exp_sb), ...]
            for t, (p, g) in enumerate(units):
                if t == 5 and n_pairs > 2:
                    queue_pair_chunks(2)
                if t == 11 and n_pairs > 3:
                    queue_pair_chunks(3)
                fat = emit_m1(p, g)
                exp_sb = emit_expmask(p, g, fat, t)
                drip(1)
                pending.append((p, g, exp_sb))
                npop = 1 if len(pending) > DSKEW else 0
                if t >= len(units) - 4 and pending:
                    npop = max(npop, 2)
                for _ in range(npop):
                    if not pending:
                        break
                    pp_, gg_, ee_ = pending.pop(0)
                    outr = emit_m2(pp_, gg_, ee_)
                    emit_norm(pp_, gg_, outr, t)
            for pp_, gg_, ee_ in pending:
                outr = emit_m2(pp_, gg_, ee_)
                emit_norm(pp_, gg_, outr, 0)

    nc.compile()
    return nc


def _get_program(mm1_mode=MM1_MODE):
    key = mm1_mode
    if key not in _PROGRAM_CACHE:
        _PROGRAM_CACHE[key] = _build_program()
    return _PROGRAM_CACHE[key]


def _prep_core_inputs(qf, kf, vf, core, mm1_mode=MM1_MODE, n_pairs=NPAIR):
    """qf,kf,vf: [64, T, E] float32 (bh-merged). Returns the core's in_map."""
    bh0 = core * BH_PER_CORE
    qkv = np.empty((n_pairs, 128, 3 * T), dtype=np.uint16)
    for p in range(n_pairs):
        for s in range(2):
            bh = bh0 + 2 * p + s
            sl = slice(s * 64, s * 64 + 64)
            qkv[p, sl, 0:T] = qf[bh].T.astype(np.float16).view(np.uint16)
            qkv[p, sl, T:2 * T] = kf[bh].T.astype(np.float16).view(np.uint16)
            # [T, E] -> [m, j(128), e] -> [j(128), m, e]; bits m-major at
            # [2T + (m*2 + s)*64 + e] per partition
            va = vf[bh].reshape(NBUCK // 2, 128, E).transpose(1, 0, 2)
            va16 = va.astype(ml_dtypes.bfloat16).view(np.uint16)
            vblk = qkv[p, :, 2 * T:3 * T].reshape(128, NBUCK // 2, 2, E)
            vblk[:, :, s, :] = va16
    return {"qkv": qkv.view(np.float16)}


def _unpack_out(res_out, core, out_full):
    """res_out: [NPAIR, 128, NBUCK//SGRP, 12, BS] f16 -> [64, T, E] f32.

    Slots 0..7 are the normalized window outputs; slots 8..11 hold the
    even windows' prev-half contributions (already normalized), which
    are added here.  w=0's aux (slot 8 of group 0) is a filler and
    skipped."""
    bh0 = core * BH_PER_CORE
    r = res_out.astype(np.float32)
    main = r[:, :, :, 0:SGRP, :]        # [p, j, g, widx, e]
    aux = r[:, :, :, SGRP:12, :]        # [p, j, g, a, e]
    main[:, :, :, 2::2, :] += aux[:, :, :, 1:, :]
    # even widx 0 of groups g>=1 corresponds to w = g*8 (even, >0): aux a=0
    main[:, :, 1:, 0, :] += aux[:, :, 1:, 0, :]
    for p in range(r.shape[0]):
        for s in range(2):
            bh = bh0 + 2 * p + s
            blk = main[p, s * 64:s * 64 + 64]  # [i, g, widx, e]
            out_full[bh] = blk.transpose(1, 2, 0, 3).reshape(T, E)


def kernel(q, k, v):
    from concourse.bass_utils import run_bass_kernel_spmd

    q = np.asarray(q, dtype=np.float32)
    k = np.asarray(k, dtype=np.float32)
    v = np.asarray(v, dtype=np.float32)
    Bq, Hq = q.shape[0], q.shape[1]
    qf = q.reshape(Bq * Hq, T, E)
    kf = k.reshape(Bq * Hq, T, E)
    vf = v.reshape(Bq * Hq, T, E)

    nc = _get_program()
    in_maps = [_prep_core_inputs(qf, kf, vf, c) for c in range(N_CORES)]
    res = run_bass_kernel_spmd(nc, in_maps, list(range(N_CORES)))

    out_full = np.empty((Bq * Hq, T, E), dtype=np.float32)
    for c in range(N_CORES):
        _unpack_out(res.results[c]["out"], c, out_full)
    return out_full.reshape(Bq, Hq, T, E)


# revision 59
# speedup vs baseline: 1.1207x; 1.1207x over previous
"""Local (bucketed) attention Bass kernel for Trainium2, 8 NeuronCores SPMD.

Problem (hardcoded): B=8, H=8, T=8192, E=64, BUCKETS=128, bucket=64,
look_backward=1, look_forward=0, causal, no 1/sqrt(E) scaling.

Sharding: batch*heads (64) split across 8 cores -> 8 bh per core,
processed as 4 "pairs"; within a pair, bh 2p is "stream A" (SBUF
partitions 0..63 of q/k) and bh 2p+1 is "stream B" (64..127).

Design (2.3x the session-start baseline; sim/HW-verified):
  - q/k shipped fp16 (single-pass mm1, ~8x the mantissa of bf16 at the
    same byte cost); v bf16; out fp16 + f32 host fixup.
  - mm1 per (stream, window w): stationary = kt 2-bucket slab in
    "parity order" (even-index bucket of the window in PE rows 0..63,
    odd in 64..127), moving = qt bucket w -> dotsT [128 j, 64 q].
    Odd w: one matmul (ascending slab is contiguous).  Even w: two
    standalone matmuls (HW requires stationary APs with ONE free dim,
    so the descending slab is not expressible).
  - exp on ACT into bf16; causal+window mask applied multiplicatively
    on DVE (2/5) / GPSIMD (3/5) with parity masks [tri;ones]/[ones;tri].
  - v parity layout va[128 j, s, m, e]: partition j<64 = bucket 2m,
    j>=64 = bucket 2m+1.  mm2 odd w: one K=128 matmul into the main
    PSUM slot.  Even w: matmuls accumulating into one PSUM region MUST
    share the lhsT partition range on HW (row-switch accumulation is an
    NRT_EXEC_UNIT_UNRECOVERABLE), so the cur half goes to the main slot
    and the prev half to an aux slot; the host adds aux terms during
    unpack.  Row sums via 1-column ones-matmuls; out tile layout per
    group = 8 main + 4 aux + 12 r columns in one 2-bank PSUM tile.
  - reciprocal + broadcast-normalize on DVE; input DMAs quarter-chunked
    and dripped one-per-unit across the SP and GPSIMD queues (the cost
    model charges DMA transfer time to the issuing engine, and a big
    DMA at a queue head blocks latency-critical work behind it);
    outputs on SP.  Software pipeline: mm2 trails mm1 by 3 units
    (PE FIFO head-of-line cover for the exp->mask round trip), units
    alternate between the two resident pairs, all 4 pairs' tiles
    resident (bufs=4) so block transitions don't stall.

HW constraints discovered (CoreSim does not model them):
  - matmul stationary AP must collapse to a single free dimension
    (no negative-stride or broadcast 2-bucket slabs).
  - all matmuls of one PSUM accumulation group (including
    start=False standalone accumulates) must use the same lhsT/rhs
    partition range; different tile_position rows crash the PE.
  - GPSIMD (Pool) tensor ops cannot access PSUM.
"""

import numpy as np
import ml_dtypes

BH_PER_CORE = 8
N_CORES = 8
T = 8192
E = 64
BS = 64  # bucket size
NBUCK = T // BS  # 128
NPAIR = BH_PER_CORE // 2  # 4
SGRP = 8   # windows per compute group
OGRP = 1   # compute groups per output DMA

MM1_MODE = "fp16-k128"  # informational only

_PROGRAM_CACHE = {}


def _build_program(n_pairs=NPAIR, nbuck=NBUCK):
    import concourse.bass as bass
    import concourse.tile as tile
    from concourse import bacc, mybir

    F32 = mybir.dt.float32
    F16 = mybir.dt.float16
    BF16 = mybir.dt.bfloat16
    Exp = mybir.ActivationFunctionType.Exp
    mult = mybir.AluOpType.mult

    nc = bacc.Bacc("TRN2", target_bir_lowering=False, debug=False,
                   num_devices=N_CORES)

    seqlen = nbuck * BS  # 8192
    # merged input: per partition fp16[0:8192]=qT, fp16[8192:16384]=kT,
    # bf16-bits[16384:24576]=va (s-major: s, m, e)
    qkv_d = nc.dram_tensor("qkv", [n_pairs, 128, 3 * seqlen], F16,
                           kind="ExternalInput").ap()
    # 12 slots per group: 8 normalized window outputs + 4 aux terms
    # (even windows' prev-half contributions; host adds them in)
    out_d = nc.dram_tensor("out", [n_pairs, 128, nbuck // SGRP, 12, BS], F16,
                           kind="ExternalOutput").ap()

    # Masks, [128 j, parity, 64 i]:
    #   parity 0 (even w): cur bucket on rows 0..63 -> [tri; ones]
    #   parity 1 (odd  w): cur bucket on rows 64..127 -> [ones; tri]
    tri = (np.arange(BS)[:, None] <= np.arange(BS)[None, :])  # keep j<=i
    mask_np = np.empty((128, 2, BS), dtype=ml_dtypes.bfloat16)
    mask_np[0:64, 0] = tri.astype(ml_dtypes.bfloat16)
    mask_np[64:128, 0] = 1.0
    mask_np[0:64, 1] = 1.0
    mask_np[64:128, 1] = tri.astype(ml_dtypes.bfloat16)
    mask_dram = nc.inline_tensor(np.ascontiguousarray(mask_np), name="winmask")
    ones_dram = nc.inline_tensor(
        np.ones((128, 1), dtype=ml_dtypes.bfloat16), name="onescol")

    ngrp = nbuck // SGRP  # 16

    with tile.TileContext(nc) as tc:
        with (
            tc.tile_pool(name="consts", bufs=1) as consts,
            tc.tile_pool(name="qkv", bufs=4) as qkvp,
            tc.tile_pool(name="expp", bufs=5) as expp,
            tc.tile_pool(name="outsb", bufs=3) as outsbp,
            tc.tile_pool(name="rp", bufs=3) as rp,
            tc.tile_pool(name="fat", bufs=2, space="PSUM") as fatp,
            tc.tile_pool(name="outr", bufs=2, space="PSUM") as outrp,
        ):
            mask_sb = consts.tile([128, 2, BS], BF16)
            nc.sync.dma_start(mask_sb[:], mask_dram.ap())
            ones_sb = consts.tile([128, 1], BF16)
            nc.sync.dma_start(ones_sb[:], ones_dram.ap())

            # per-pair views, filled lazily when the pair's DMA is issued
            views = {}

            def _mk_tiles(p):
                kt_sb = qkvp.tile([128, seqlen], F16, tag="kt", name="kt_sb")
                qt_sb = qkvp.tile([128, seqlen], F16, tag="qt", name="qt_sb")
                va_sb = qkvp.tile([128, seqlen], F16, tag="va", name="va_sb")
                qt_r = qt_sb[:].rearrange("p (b x) -> p b x", x=BS)
                kt_r = kt_sb[:].rearrange("p (b x) -> p b x", x=BS)
                # va bits are m-major: [m, s, e] per partition
                va_r = va_sb[:].bitcast(BF16).rearrange(
                    "p (m s e) -> p s m e", s=2, e=BS)
                views[p] = (qt_r, kt_r, va_r)
                return qt_sb, kt_sb, va_sb

            _drip = {0: [], 1: []}  # queue -> list of (tile_slice, dram_slice)

            def queue_pair_chunks(p, nchunk=8):
                # quarter-chunks of kt, qt, va appended to the two drip
                # queues (kt/qt lead va by construction order)
                qt_sb, kt_sb, va_sb = _mk_tiles(p)
                plan = [(kt_sb, seqlen), (qt_sb, 0), (va_sb, 2 * seqlen)]
                qc = seqlen // nchunk
                qi = p % 2
                for ci in range(nchunk):
                    lo = ci * qc
                    for tl, base in plan:
                        _drip[qi].append(
                            (tl[:, lo:lo + qc],
                             qkv_d[p, :, base + lo:base + lo + qc]))
                        qi ^= 1

            def drip(n=1):
                qs = (nc.sync, nc.gpsimd)
                for qi in (0, 1):
                    for _ in range(n):
                        if _drip[qi]:
                            tl, dr = _drip[qi].pop(0)
                            qs[qi].dma_start(tl, dr)

            def load_block0_start():
                t0 = _mk_tiles(0)
                t1 = _mk_tiles(1)
                tensors = [
                    (0, 0, t0[1], seqlen),       # SP:   kt0
                    (1, 0, t0[0], 0),            # Pool: qt0
                    (1, 1, t1[1], seqlen),       # Pool: kt1
                    (0, 1, t1[0], 0),            # SP:   qt1
                    (1, 0, t0[2], 2 * seqlen),   # Pool: va0
                    (0, 1, t1[2], 2 * seqlen),   # SP:   va1
                ]
                # first quarter of everything lands up front (two eighth
                # rounds for kt/qt, one quarter for va)
                qs = (nc.sync, nc.gpsimd)
                e8 = seqlen // 8
                for lo, hi in ((0, e8), (e8, 2 * e8)):
                    for qi, pr, tl, base in tensors[:4]:
                        qs[qi].dma_start(tl[:, lo:hi],
                                         qkv_d[pr, :, base + lo:base + hi])
                for qi, pr, tl, base in tensors[4:]:
                    qs[qi].dma_start(tl[:, 0:2 * e8],
                                     qkv_d[pr, :, base:base + 2 * e8])
                # remaining 3/4, quarter-chunks into the drip queues
                qc = seqlen // 8
                for ci in range(2, 8):
                    lo = ci * qc
                    for qi, pr, tl, base in tensors:
                        _drip[qi].append(
                            (tl[:, lo:lo + qc],
                             qkv_d[pr, :, base + lo:base + lo + qc]))

            def emit_m1(p, g):
                """mm1 group -> fat psum tile; returns (fat, exp_sb)."""
                qt_r, kt_r, va_r = views[p]
                w0 = g * SGRP
                fat = fatp.tile([128, 2, SGRP, BS], F32, tag="fat")
                for s in range(2):
                    sp = s * 64
                    for widx in range(SGRP):
                        w = w0 + widx
                        rhs = qt_r[sp:sp + 64, w, :]
                        if w == 0 or w % 2 == 1:
                            # ascending 2-bucket slab (contiguous, single
                            # free dim).  For w=0 rows 64..127 hold
                            # never-read bucket-1 scores (finite filler).
                            wl = max(w - 1, 0)
                            nc.tensor.matmul(
                                fat[:, s, widx, :],
                                lhsT=kt_r[sp:sp + 64, wl:wl + 2, :],
                                rhs=rhs, start=True, stop=True)
                        else:
                            # even w: cur bucket -> rows 0..63, prev ->
                            # rows 64..127 (HW stationary APs must be a
                            # single free dim, so two matmuls)
                            nc.tensor.matmul(
                                fat[0:64, s, widx, :],
                                lhsT=kt_r[sp:sp + 64, w, :],
                                rhs=rhs, start=True, stop=True)
                            nc.tensor.matmul(
                                fat[64:128, s, widx, :],
                                lhsT=kt_r[sp:sp + 64, w - 1, :],
                                rhs=rhs, start=True, stop=True)
                return fat

            def emit_expmask(p, g, fat, t):
                exp_sb = expp.tile([128, 2, SGRP, BS], BF16, tag="exp")
                nc.scalar.activation(exp_sb[:], fat[:], Exp)
                ev = exp_sb[:].rearrange("p s (a q) i -> p s a q i", q=2)
                eng = nc.vector if t % 3 == 0 else nc.gpsimd
                eng.tensor_tensor(
                    ev, ev,
                    mask_sb[:, None, None, :, :].to_broadcast(
                        (128, 2, SGRP // 2, 2, BS)),
                    mult,
                )
                return exp_sb

            def emit_m2(p, g, exp_sb):
                # outr layout (f32 cols): [0:512) = 8 main slots,
                # [512:768) = 4 aux slots (even windows' prev halves),
                # [768:780) = 12 row-sum columns.  All matmuls standalone
                # or same-partition-range groups (HW requirement).
                qt_r, kt_r, va_r = views[p]
                w0 = g * SGRP
                outr = outrp.tile([128, 784], F32, tag="outr")
                main = outr[:, 0:512].rearrange("p (w e) -> p w e", e=BS)
                aux = outr[:, 512:768].rearrange("p (w e) -> p w e", e=BS)
                rcol = outr[:, 768:780]
                for s in range(2):
                    ob0 = s * 64
                    for widx in range(SGRP):
                        w = w0 + widx
                        ex = exp_sb[:, s, widx, :]
                        o = main[ob0:ob0 + 64, widx, :]
                        ro = rcol[ob0:ob0 + 64, widx:widx + 1]
                        if w == 0:
                            nc.tensor.matmul(
                                o, lhsT=ex[0:64, :],
                                rhs=va_r[0:64, s, 0, :],
                                start=True, stop=True)
                            nc.tensor.matmul(
                                ro, lhsT=ex[0:64, :],
                                rhs=ones_sb[0:64, :],
                                start=True, stop=True)
                            # fill aux slot 0 / r col 8 so the batched
                            # normalize reads initialized psum (host
                            # ignores aux for w=0)
                            nc.tensor.matmul(
                                aux[ob0:ob0 + 64, 0, :], lhsT=ex[0:64, :],
                                rhs=va_r[0:64, s, 0, :],
                                start=True, stop=True)
                            nc.tensor.matmul(
                                rcol[ob0:ob0 + 64, 8:9], lhsT=ex[0:64, :],
                                rhs=ones_sb[0:64, :],
                                start=True, stop=True)
                        elif w % 2 == 1:
                            m = (w - 1) // 2
                            nc.tensor.matmul(
                                o, lhsT=ex, rhs=va_r[:, s, m, :],
                                start=True, stop=True)
                            nc.tensor.matmul(
                                ro, lhsT=ex, rhs=ones_sb[:],
                                start=True, stop=True)
                        else:
                            m = w // 2
                            a = widx // 2
                            nc.tensor.matmul(
                                o, lhsT=ex[0:64, :],
                                rhs=va_r[0:64, s, m, :],
                                start=True, stop=True)
                            nc.tensor.matmul(
                                aux[ob0:ob0 + 64, a, :],
                                lhsT=ex[64:128, :],
                                rhs=va_r[64:128, s, m - 1, :],
                                start=True, stop=True)
                            nc.tensor.matmul(
                                ro, lhsT=ex, rhs=ones_sb[:],
                                start=True, stop=True)
                            nc.tensor.matmul(
                                rcol[ob0:ob0 + 64, 8 + a:9 + a],
                                lhsT=ex, rhs=ones_sb[:],
                                start=True, stop=True)
                return outr

            obs = {}
            _oblk = [0]

            def emit_norm(p, g, outr, scale_idx):
                r_sb = rp.tile([128, 12], F32, tag="r")
                nc.vector.reciprocal(r_sb[:], outr[:, 768:780])
                key = p % 2
                if obs.get(key) is None:
                    ob = outsbp.tile([128, OGRP, 12, BS], F16, tag="ob",
                                     name="ob")
                    obs[key] = ob
                ob = obs[key]
                gslot = g % OGRP
                ov = outr[:, 0:768].rearrange("p (w e) -> p w e", e=BS)
                nc.vector.tensor_tensor(
                    ob[:, gslot, :, :],
                    ov,
                    r_sb[:, :, None].to_broadcast((128, 12, BS)),
                    mult,
                )
                if gslot == OGRP - 1:
                    glo = g - OGRP + 1
                    eng = nc.sync
                    _oblk[0] += 1
                    eng.dma_start(out_d[p, :, glo:glo + OGRP, :, :], ob[:])
                    obs[key] = None

            # flat unit list: two pair-blocks, pairs interleaved inside
            units = []
            for blk in range(n_pairs // 2):
                for g in range(ngrp):
                    for pp in range(2):
                        units.append((2 * blk + pp, g))

            load_block0_start()
            DSKEW = 3
            pending = []  # [(p, g, exp_sb), ...]
            for t, (p, g) in enumerate(units):
                if t == 5 and n_pairs > 2:
                    queue_pair_chunks(2)
                if t == 11 and n_pairs > 3:
                    queue_pair_chunks(3)
                fat = emit_m1(p, g)
                exp_sb = emit_expmask(p, g, fat, t)
                drip(1)
                pending.append((p, g, exp_sb))
                npop = 1 if len(pending) > DSKEW else 0
                if t >= len(units) - 4 and pending:
                    npop = max(npop, 2)
                for _ in range(npop):
                    if not pending:
                        break
                    pp_, gg_, ee_ = pending.pop(0)
                    outr = emit_m2(pp_, gg_, ee_)
                    emit_norm(pp_, gg_, outr, t)
            for pp_, gg_, ee_ in pending:
                outr = emit_m2(pp_, gg_, ee_)
                emit_norm(pp_, gg_, outr, 0)

    nc.compile()
    return nc


def _get_program(mm1_mode=MM1_MODE):
    key = mm1_mode
    if key not in _PROGRAM_CACHE:
        _PROGRAM_CACHE[key] = _build_program()
    return _PROGRAM_CACHE[key]


def _prep_core_inputs(qf, kf, vf, core, mm1_mode=MM1_MODE, n_pairs=NPAIR):
    """qf,kf,vf: [64, T, E] float32 (bh-merged). Returns the core's in_map."""
    bh0 = core * BH_PER_CORE
    qkv = np.empty((n_pairs, 128, 3 * T), dtype=np.uint16)
    for p in range(n_pairs):
        for s in range(2):
            bh = bh0 + 2 * p + s
            sl = slice(s * 64, s * 64 + 64)
            qkv[p, sl, 0:T] = qf[bh].T.astype(np.float16).view(np.uint16)
            qkv[p, sl, T:2 * T] = kf[bh].T.astype(np.float16).view(np.uint16)
            # [T, E] -> [m, j(128), e] -> [j(128), m, e]; bits m-major at
            # [2T + (m*2 + s)*64 + e] per partition
            va = vf[bh].reshape(NBUCK // 2, 128, E).transpose(1, 0, 2)
            va16 = va.astype(ml_dtypes.bfloat16).view(np.uint16)
            vblk = qkv[p, :, 2 * T:3 * T].reshape(128, NBUCK // 2, 2, E)
            vblk[:, :, s, :] = va16
    return {"qkv": qkv.view(np.float16)}


def _unpack_out(res_out, core, out_full):
    """res_out: [NPAIR, 128, NBUCK//SGRP, 12, BS] f16 -> [64, T, E] f32.

    Slots 0..7 are the normalized window outputs; slots 8..11 hold the
    even windows' prev-half contributions (already normalized), which
    are added here.  w=0's aux (slot 8 of group 0) is a filler and
    skipped."""
    bh0 = core * BH_PER_CORE
    r = res_out.astype(np.float32)
    main = r[:, :, :, 0:SGRP, :]        # [p, j, g, widx, e]
    aux = r[:, :, :, SGRP:12, :]        # [p, j, g, a, e]
    main[:, :, :, 2::2, :] += aux[:, :, :, 1:, :]
    # even widx 0 of groups g>=1 corresponds to w = g*8 (even, >0): aux a=0
    main[:, :, 1:, 0, :] += aux[:, :, 1:, 0, :]
    for p in range(r.shape[0]):
        for s in range(2):
            bh = bh0 + 2 * p + s
            blk = main[p, s * 64:s * 64 + 64]  # [i, g, widx, e]
            out_full[bh] = blk.transpose(1, 2, 0, 3).reshape(T, E)


def kernel(q, k, v):
    from concourse.bass_utils import run_bass_kernel_spmd

    q = np.asarray(q, dtype=np.float32)
    k = np.asarray(k, dtype=np.float32)
    v = np.asarray(v, dtype=np.float32)
    Bq, Hq = q.shape[0], q.shape[1]
    qf = q.reshape(Bq * Hq, T, E)
    kf = k.reshape(Bq * Hq, T, E)
    vf = v.reshape(Bq * Hq, T, E)

    nc = _get_program()
    in_maps = [_prep_core_inputs(qf, kf, vf, c) for c in range(N_CORES)]
    res = run_bass_kernel_spmd(nc, in_maps, list(range(N_CORES)))

    out_full = np.empty((Bq * Hq, T, E), dtype=np.float32)
    for c in range(N_CORES):
        _unpack_out(res.results[c]["out"], c, out_full)
    return out_full.reshape(Bq, Hq, T, E)


# revision 63
# speedup vs baseline: 1.1286x; 1.0071x over previous
"""Local (bucketed) attention Bass kernel for Trainium2, 8 NeuronCores SPMD.

Problem (hardcoded): B=8, H=8, T=8192, E=64, BUCKETS=128, bucket=64,
look_backward=1, look_forward=0, causal, no 1/sqrt(E) scaling.

Sharding: batch*heads (64) split across 8 cores -> 8 bh per core,
processed as 4 "pairs"; within a pair, bh 2p is "stream A" (SBUF
partitions 0..63 of q/k) and bh 2p+1 is "stream B" (64..127).

Design (2.6x the session-start baseline; sim/HW-verified):
  - q/k shipped fp16 (single-pass mm1, ~8x the mantissa of bf16 at the
    same byte cost); v bf16; out fp16 + f32 host fixup.
  - mm1 per (stream, window w): stationary = kt 2-bucket slab in
    "parity order" (even-index bucket of the window in PE rows 0..63,
    odd in 64..127), moving = qt bucket w -> dotsT [128 j, 64 q].
    Odd w: one matmul (ascending slab is contiguous).  Even w: two
    standalone matmuls (HW requires stationary APs with ONE free dim,
    so the descending slab is not expressible).
  - exp on ACT into bf16; causal+window mask applied multiplicatively
    on DVE (1/3) / GPSIMD (2/3) with parity masks [tri;ones]/[ones;tri].
  - v parity layout va[128 j, s, m, e]: partition j<64 = bucket 2m,
    j>=64 = bucket 2m+1.  mm2 odd w: one K=128 matmul into the main
    PSUM slot.  Even w: matmuls accumulating into one PSUM region MUST
    share the lhsT partition range on HW (row-switch accumulation is an
    NRT_EXEC_UNIT_UNRECOVERABLE), so the cur half goes to the main slot
    and the prev half to an aux slot; the host adds aux terms during
    unpack.  Row sums via 1-column ones-matmuls; out tile layout per
    group = 8 main + 4 aux + 12 r columns in one 2-bank PSUM tile.
  - reciprocal + broadcast-normalize on DVE; input DMAs eighth-chunked
    (~0.75 us each) and dripped one-per-unit across the SP and GPSIMD
    queues (the cost model charges DMA transfer time to the issuing
    engine, and any DMA at a queue head blocks latency-critical masks
    behind it -- chunk size directly sets the mask-wait tail);
    outputs on SP.  Software pipeline: mm2 trails mm1 by 2 units
    (PE FIFO head-of-line cover for the exp->mask round trip), units
    alternate between the two resident pairs, all 4 pairs' tiles
    resident (bufs=4) so block transitions don't stall.

HW constraints discovered (CoreSim does not model them):
  - matmul stationary AP must collapse to a single free dimension
    (no negative-stride or broadcast 2-bucket slabs).
  - all matmuls of one PSUM accumulation group (including
    start=False standalone accumulates) must use the same lhsT/rhs
    partition range; different tile_position rows crash the PE.
  - GPSIMD (Pool) tensor ops cannot access PSUM.
"""

import numpy as np
import ml_dtypes

BH_PER_CORE = 8
N_CORES = 8
T = 8192
E = 64
BS = 64  # bucket size
NBUCK = T // BS  # 128
NPAIR = BH_PER_CORE // 2  # 4
SGRP = 8   # windows per compute group
OGRP = 1   # compute groups per output DMA

MM1_MODE = "fp16-k128"  # informational only

_PROGRAM_CACHE = {}


def _build_program(n_pairs=NPAIR, nbuck=NBUCK):
    import concourse.bass as bass
    import concourse.tile as tile
    from concourse import bacc, mybir

    F32 = mybir.dt.float32
    F16 = mybir.dt.float16
    BF16 = mybir.dt.bfloat16
    Exp = mybir.ActivationFunctionType.Exp
    mult = mybir.AluOpType.mult

    nc = bacc.Bacc("TRN2", target_bir_lowering=False, debug=False,
                   num_devices=N_CORES)

    seqlen = nbuck * BS  # 8192
    # merged input: per partition fp16[0:8192]=qT, fp16[8192:16384]=kT,
    # bf16-bits[16384:24576]=va (s-major: s, m, e)
    qkv_d = nc.dram_tensor("qkv", [n_pairs, 128, 3 * seqlen], F16,
                           kind="ExternalInput").ap()
    # 12 slots per group: 8 normalized window outputs + 4 aux terms
    # (even windows' prev-half contributions; host adds them in)
    out_d = nc.dram_tensor("out", [n_pairs, 128, nbuck // SGRP, 12, BS], F16,
                           kind="ExternalOutput").ap()

    # Masks, [128 j, parity, 64 i]:
    #   parity 0 (even w): cur bucket on rows 0..63 -> [tri; ones]
    #   parity 1 (odd  w): cur bucket on rows 64..127 -> [ones; tri]
    tri = (np.arange(BS)[:, None] <= np.arange(BS)[None, :])  # keep j<=i
    mask_np = np.empty((128, 2, BS), dtype=ml_dtypes.bfloat16)
    mask_np[0:64, 0] = tri.astype(ml_dtypes.bfloat16)
    mask_np[64:128, 0] = 1.0
    mask_np[0:64, 1] = 1.0
    mask_np[64:128, 1] = tri.astype(ml_dtypes.bfloat16)
    mask_dram = nc.inline_tensor(np.ascontiguousarray(mask_np), name="winmask")
    ones_dram = nc.inline_tensor(
        np.ones((128, 1), dtype=ml_dtypes.bfloat16), name="onescol")

    ngrp = nbuck // SGRP  # 16

    with tile.TileContext(nc) as tc:
        with (
            tc.tile_pool(name="consts", bufs=1) as consts,
            tc.tile_pool(name="qkv", bufs=4) as qkvp,
            tc.tile_pool(name="expp", bufs=5) as expp,
            tc.tile_pool(name="outsb", bufs=3) as outsbp,
            tc.tile_pool(name="rp", bufs=3) as rp,
            tc.tile_pool(name="fat", bufs=2, space="PSUM") as fatp,
            tc.tile_pool(name="outr", bufs=2, space="PSUM") as outrp,
        ):
            mask_sb = consts.tile([128, 2, BS], BF16)
            nc.sync.dma_start(mask_sb[:], mask_dram.ap())
            ones_sb = consts.tile([128, 1], BF16)
            nc.sync.dma_start(ones_sb[:], ones_dram.ap())

            # per-pair views, filled lazily when the pair's DMA is issued
            views = {}

            def _mk_tiles(p):
                kt_sb = qkvp.tile([128, seqlen], F16, tag="kt", name="kt_sb")
                qt_sb = qkvp.tile([128, seqlen], F16, tag="qt", name="qt_sb")
                va_sb = qkvp.tile([128, seqlen], F16, tag="va", name="va_sb")
                qt_r = qt_sb[:].rearrange("p (b x) -> p b x", x=BS)
                kt_r = kt_sb[:].rearrange("p (b x) -> p b x", x=BS)
                # va bits are m-major: [m, s, e] per partition
                va_r = va_sb[:].bitcast(BF16).rearrange(
                    "p (m s e) -> p s m e", s=2, e=BS)
                views[p] = (qt_r, kt_r, va_r)
                return qt_sb, kt_sb, va_sb

            _drip = {0: [], 1: []}  # queue -> list of (tile_slice, dram_slice)

            def queue_pair_chunks(p, nchunk=8):
                # quarter-chunks of kt, qt, va appended to the two drip
                # queues (kt/qt lead va by construction order)
                qt_sb, kt_sb, va_sb = _mk_tiles(p)
                plan = [(kt_sb, seqlen), (qt_sb, 0), (va_sb, 2 * seqlen)]
                qc = seqlen // nchunk
                qi = p % 2
                for ci in range(nchunk):
                    lo = ci * qc
                    for tl, base in plan:
                        _drip[qi].append(
                            (tl[:, lo:lo + qc],
                             qkv_d[p, :, base + lo:base + lo + qc]))
                        qi ^= 1

            def drip(n=1):
                qs = (nc.sync, nc.gpsimd)
                for qi in (0, 1):
                    for _ in range(n):
                        if _drip[qi]:
                            tl, dr = _drip[qi].pop(0)
                            qs[qi].dma_start(tl, dr)

            def load_block0_start():
                t0 = _mk_tiles(0)
                t1 = _mk_tiles(1)
                tensors = [
                    (0, 0, t0[1], seqlen),       # SP:   kt0
                    (1, 0, t0[0], 0),            # Pool: qt0
                    (1, 1, t1[1], seqlen),       # Pool: kt1
                    (0, 1, t1[0], 0),            # SP:   qt1
                    (1, 0, t0[2], 2 * seqlen),   # Pool: va0
                    (0, 1, t1[2], 2 * seqlen),   # SP:   va1
                ]
                # first quarter of everything lands up front (two eighth
                # rounds for kt/qt, one quarter for va)
                qs = (nc.sync, nc.gpsimd)
                e8 = seqlen // 8
                for lo, hi in ((0, e8), (e8, 2 * e8)):
                    for qi, pr, tl, base in tensors[:4]:
                        qs[qi].dma_start(tl[:, lo:hi],
                                         qkv_d[pr, :, base + lo:base + hi])
                for qi, pr, tl, base in tensors[4:]:
                    qs[qi].dma_start(tl[:, 0:2 * e8],
                                     qkv_d[pr, :, base:base + 2 * e8])
                # remaining 3/4, quarter-chunks into the drip queues
                qc = seqlen // 8
                for ci in range(2, 8):
                    lo = ci * qc
                    for qi, pr, tl, base in tensors:
                        _drip[qi].append(
                            (tl[:, lo:lo + qc],
                             qkv_d[pr, :, base + lo:base + lo + qc]))

            def emit_m1(p, g):
                """mm1 group -> fat psum tile; returns (fat, exp_sb)."""
                qt_r, kt_r, va_r = views[p]
                w0 = g * SGRP
                fat = fatp.tile([128, 2, SGRP, BS], F32, tag="fat")
                for s in range(2):
                    sp = s * 64
                    for widx in range(SGRP):
                        w = w0 + widx
                        rhs = qt_r[sp:sp + 64, w, :]
                        if w == 0 or w % 2 == 1:
                            # ascending 2-bucket slab (contiguous, single
                            # free dim).  For w=0 rows 64..127 hold
                            # never-read bucket-1 scores (finite filler).
                            wl = max(w - 1, 0)
                            nc.tensor.matmul(
                                fat[:, s, widx, :],
                                lhsT=kt_r[sp:sp + 64, wl:wl + 2, :],
                                rhs=rhs, start=True, stop=True)
                        else:
                            # even w: cur bucket -> rows 0..63, prev ->
                            # rows 64..127 (HW stationary APs must be a
                            # single free dim, so two matmuls)
                            nc.tensor.matmul(
                                fat[0:64, s, widx, :],
                                lhsT=kt_r[sp:sp + 64, w, :],
                                rhs=rhs, start=True, stop=True)
                            nc.tensor.matmul(
                                fat[64:128, s, widx, :],
                                lhsT=kt_r[sp:sp + 64, w - 1, :],
                                rhs=rhs, start=True, stop=True)
                return fat

            def emit_expmask(p, g, fat, t):
                exp_sb = expp.tile([128, 2, SGRP, BS], BF16, tag="exp")
                nc.scalar.activation(exp_sb[:], fat[:], Exp)
                ev = exp_sb[:].rearrange("p s (a q) i -> p s a q i", q=2)
                eng = nc.vector if t % 3 == 0 else nc.gpsimd
                eng.tensor_tensor(
                    ev, ev,
                    mask_sb[:, None, None, :, :].to_broadcast(
                        (128, 2, SGRP // 2, 2, BS)),
                    mult,
                )
                return exp_sb

            def emit_m2(p, g, exp_sb):
                # outr layout (f32 cols): [0:512) = 8 main slots,
                # [512:768) = 4 aux slots (even windows' prev halves),
                # [768:780) = 12 row-sum columns.  All matmuls standalone
                # or same-partition-range groups (HW requirement).
                qt_r, kt_r, va_r = views[p]
                w0 = g * SGRP
                outr = outrp.tile([128, 784], F32, tag="outr")
                main = outr[:, 0:512].rearrange("p (w e) -> p w e", e=BS)
                aux = outr[:, 512:768].rearrange("p (w e) -> p w e", e=BS)
                rcol = outr[:, 768:780]
                for s in range(2):
                    ob0 = s * 64
                    for widx in range(SGRP):
                        w = w0 + widx
                        ex = exp_sb[:, s, widx, :]
                        o = main[ob0:ob0 + 64, widx, :]
                        ro = rcol[ob0:ob0 + 64, widx:widx + 1]
                        if w == 0:
                            nc.tensor.matmul(
                                o, lhsT=ex[0:64, :],
                                rhs=va_r[0:64, s, 0, :],
                                start=True, stop=True)
                            nc.tensor.matmul(
                                ro, lhsT=ex[0:64, :],
                                rhs=ones_sb[0:64, :],
                                start=True, stop=True)
                            # fill aux slot 0 / r col 8 so the batched
                            # normalize reads initialized psum (host
                            # ignores aux for w=0)
                            nc.tensor.matmul(
                                aux[ob0:ob0 + 64, 0, :], lhsT=ex[0:64, :],
                                rhs=va_r[0:64, s, 0, :],
                                start=True, stop=True)
                            nc.tensor.matmul(
                                rcol[ob0:ob0 + 64, 8:9], lhsT=ex[0:64, :],
                                rhs=ones_sb[0:64, :],
                                start=True, stop=True)
                        elif w % 2 == 1:
                            m = (w - 1) // 2
                            nc.tensor.matmul(
                                o, lhsT=ex, rhs=va_r[:, s, m, :],
                                start=True, stop=True)
                            nc.tensor.matmul(
                                ro, lhsT=ex, rhs=ones_sb[:],
                                start=True, stop=True)
                        else:
                            m = w // 2
                            a = widx // 2
                            nc.tensor.matmul(
                                o, lhsT=ex[0:64, :],
                                rhs=va_r[0:64, s, m, :],
                                start=True, stop=True)
                            nc.tensor.matmul(
                                aux[ob0:ob0 + 64, a, :],
                                lhsT=ex[64:128, :],
                                rhs=va_r[64:128, s, m - 1, :],
                                start=True, stop=True)
                            nc.tensor.matmul(
                                ro, lhsT=ex, rhs=ones_sb[:],
                                start=True, stop=True)
                            nc.tensor.matmul(
                                rcol[ob0:ob0 + 64, 8 + a:9 + a],
                                lhsT=ex, rhs=ones_sb[:],
                                start=True, stop=True)
                return outr

            obs = {}
            _oblk = [0]

            def emit_norm(p, g, outr, scale_idx):
                r_sb = rp.tile([128, 12], F32, tag="r")
                nc.vector.reciprocal(r_sb[:], outr[:, 768:780])
                key = p % 2
                if obs.get(key) is None:
                    ob = outsbp.tile([128, OGRP, 12, BS], F16, tag="ob",
                                     name="ob")
                    obs[key] = ob
                ob = obs[key]
                gslot = g % OGRP
                ov = outr[:, 0:768].rearrange("p (w e) -> p w e", e=BS)
                nc.vector.tensor_tensor(
                    ob[:, gslot, :, :],
                    ov,
                    r_sb[:, :, None].to_broadcast((128, 12, BS)),
                    mult,
                )
                if gslot == OGRP - 1:
                    glo = g - OGRP + 1
                    eng = nc.sync
                    _oblk[0] += 1
                    eng.dma_start(out_d[p, :, glo:glo + OGRP, :, :], ob[:])
                    obs[key] = None

            # flat unit list: two pair-blocks, pairs interleaved inside
            units = []
            for blk in range(n_pairs // 2):
                for g in range(ngrp):
                    for pp in range(2):
                        units.append((2 * blk + pp, g))

            load_block0_start()
            DSKEW = 2
            pending = []  # [(p, g, exp_sb), ...]
            for t, (p, g) in enumerate(units):
                if t == 5 and n_pairs > 2:
                    queue_pair_chunks(2)
                if t == 11 and n_pairs > 3:
                    queue_pair_chunks(3)
                fat = emit_m1(p, g)
                exp_sb = emit_expmask(p, g, fat, t)
                drip(1)
                pending.append((p, g, exp_sb))
                npop = 1 if len(pending) > DSKEW else 0
                if t >= len(units) - 4 and pending:
                    npop = max(npop, 2)
                for _ in range(npop):
                    if not pending:
                        break
                    pp_, gg_, ee_ = pending.pop(0)
                    outr = emit_m2(pp_, gg_, ee_)
                    emit_norm(pp_, gg_, outr, t)
            for pp_, gg_, ee_ in pending:
                outr = emit_m2(pp_, gg_, ee_)
                emit_norm(pp_, gg_, outr, 0)

    nc.compile()
    return nc


def _get_program(mm1_mode=MM1_MODE):
    key = mm1_mode
    if key not in _PROGRAM_CACHE:
        _PROGRAM_CACHE[key] = _build_program()
    return _PROGRAM_CACHE[key]


def _prep_core_inputs(qf, kf, vf, core, mm1_mode=MM1_MODE, n_pairs=NPAIR):
    """qf,kf,vf: [64, T, E] float32 (bh-merged). Returns the core's in_map."""
    bh0 = core * BH_PER_CORE
    qkv = np.empty((n_pairs, 128, 3 * T), dtype=np.uint16)
    for p in range(n_pairs):
        for s in range(2):
            bh = bh0 + 2 * p + s
            sl = slice(s * 64, s * 64 + 64)
            qkv[p, sl, 0:T] = qf[bh].T.astype(np.float16).view(np.uint16)
            qkv[p, sl, T:2 * T] = kf[bh].T.astype(np.float16).view(np.uint16)
            # [T, E] -> [m, j(128), e] -> [j(128), m, e]; bits m-major at
            # [2T + (m*2 + s)*64 + e] per partition
            va = vf[bh].reshape(NBUCK // 2, 128, E).transpose(1, 0, 2)
            va16 = va.astype(ml_dtypes.bfloat16).view(np.uint16)
            vblk = qkv[p, :, 2 * T:3 * T].reshape(128, NBUCK // 2, 2, E)
            vblk[:, :, s, :] = va16
    return {"qkv": qkv.view(np.float16)}


def _unpack_out(res_out, core, out_full):
    """res_out: [NPAIR, 128, NBUCK//SGRP, 12, BS] f16 -> [64, T, E] f32.

    Slots 0..7 are the normalized window outputs; slots 8..11 hold the
    even windows' prev-half contributions (already normalized), which
    are added here.  w=0's aux (slot 8 of group 0) is a filler and
    skipped."""
    bh0 = core * BH_PER_CORE
    r = res_out.astype(np.float32)
    main = r[:, :, :, 0:SGRP, :]        # [p, j, g, widx, e]
    aux = r[:, :, :, SGRP:12, :]        # [p, j, g, a, e]
    main[:, :, :, 2::2, :] += aux[:, :, :, 1:, :]
    # even widx 0 of groups g>=1 corresponds to w = g*8 (even, >0): aux a=0
    main[:, :, 1:, 0, :] += aux[:, :, 1:, 0, :]
    for p in range(r.shape[0]):
        for s in range(2):
            bh = bh0 + 2 * p + s
            blk = main[p, s * 64:s * 64 + 64]  # [i, g, widx, e]
            out_full[bh] = blk.transpose(1, 2, 0, 3).reshape(T, E)


def kernel(q, k, v):
    from concourse.bass_utils import run_bass_kernel_spmd

    q = np.asarray(q, dtype=np.float32)
    k = np.asarray(k, dtype=np.float32)
    v = np.asarray(v, dtype=np.float32)
    Bq, Hq = q.shape[0], q.shape[1]
    qf = q.reshape(Bq * Hq, T, E)
    kf = k.reshape(Bq * Hq, T, E)
    vf = v.reshape(Bq * Hq, T, E)

    nc = _get_program()
    in_maps = [_prep_core_inputs(qf, kf, vf, c) for c in range(N_CORES)]
    res = run_bass_kernel_spmd(nc, in_maps, list(range(N_CORES)))

    out_full = np.empty((Bq * Hq, T, E), dtype=np.float32)
    for c in range(N_CORES):
        _unpack_out(res.results[c]["out"], c, out_full)
    return out_full.reshape(Bq, Hq, T, E)
